# revision 10
# baseline (speedup 1.0000x reference)
"""GNN message passing (src_mul_edge + segment_sum) on 8 Trainium2 cores. v7.

out[n] = sum_{e : dst[e]==n} e_att[e] * src_emb[src[e]]

PE scatter-matmul + per-core token-merge design:
  * Per core, rows are PAIRED by a permutation sigma chosen so that rows
    co-used by the same node-group share one 256-byte pair-token; each
    gathered token then serves edges of BOTH its rows (even/odd halves are
    weighted independently), cutting gather descriptors/bytes ~20%.
  * embP_c [25088, 128] fp16: token t = (sigma_2t | sigma_2t+1) rows.
  * Node groups: <=32 nodes, <=384 post-merge slots (3 blocks of 128);
    uniform 3-block groups -> one shared program for all 8 cores.
  * dma_gather(transpose=False) round-robin over all 4 SWDGE queues.
  * Per 128-slot block one fused matmul on the tensor engine:
      psum[64, 128] += [A_ev | A_od][128, 64].T @ msg[128, 128]
    Valid quadrants: rows 0:32 x cols 0:64 (even halves), rows 32:64 x
    cols 64:128 (odd halves); cross-quadrants are ignored garbage.
    PSUM accumulates over the group's 3 blocks.
  * Supertile = 3 groups: scalar-engine copy + DVE add fold the quadrants
    into stage[96, 64]; one DMA per supertile to DRAM. Host unpermutes.
"""

import numpy as np

N_SRC = 50000
N_DST = 50000
D = 64
N_CORES = 8
P = 128
NPAIR = 25088
GROUP_SLOTS = 384          # 3 blocks of 128
GROUP_NODES = 32           # PSUM strip width
BLOCKS_PER_GROUP = 3
CHUNK_BLOCKS = 16
PRIME_BLOCKS = 4           # first 4 chunks are 4 blocks each

_cache: dict = {}

TRACE = False
TRACE_DIR = None
LAST_EXEC_NS = None


def _wrap_idx(idx_flat):
    w = idx_flat.reshape(-1, 16).T
    return np.tile(w, (8, 1))


def _plan_core(nodes_c, deg, dst_idx, src_idx, edge_ids_c):
    """Group one core's nodes and build its sigma pairing.

    Returns (groups, slot_tok, tok_of_row, half_of_row).
    """
    eo = edge_ids_c[np.argsort(dst_idx[edge_ids_c], kind="stable")]
    ds = dst_idx[eo]
    ss = src_idx[eo]
    starts = np.searchsorted(ds, nodes_c)
    ends = np.searchsorted(ds, nodes_c, side="right")
    node_rows = {int(n): ss[a:b] for n, a, b in zip(nodes_c, starts, ends)}

    nodes_sorted = nodes_c[np.argsort(-deg[nodes_c], kind="stable")]

    tok_of_row = {}
    half_of_row = {}
    state = {"n_tok": 0, "pending": None}

    seen = set()
    groups = []
    group_rows = []
    cur_rows = set()
    cur_nodes = []

    def cur_slots(rows):
        ext = sum(1 for r in rows if r in seen)
        fa = len(rows) - ext
        return ext + (fa + 1) // 2

    def close_group():
        nonlocal cur_rows, cur_nodes
        fa = [r for r in cur_rows if r not in seen]
        i = 0
        while i + 1 < len(fa):
            r1, r2 = fa[i], fa[i + 1]
            tok_of_row[r1] = state["n_tok"]
            half_of_row[r1] = 0
            tok_of_row[r2] = state["n_tok"]
            half_of_row[r2] = 1
            state["n_tok"] += 1
            i += 2
        if i < len(fa):
            r = fa[i]
            if state["pending"] is not None:
                tok_of_row[r] = tok_of_row[state["pending"]]
                half_of_row[r] = 1
                state["pending"] = None
            else:
                tok_of_row[r] = state["n_tok"]
                half_of_row[r] = 0
                state["pending"] = r
                state["n_tok"] += 1
        seen.update(fa)
        groups.append(cur_nodes)
        group_rows.append(cur_rows)
        cur_rows = set()
        cur_nodes = []

    for n in nodes_sorted:
        rows = set(int(r) for r in node_rows[int(n)])
        test = cur_rows | rows
        if cur_nodes and (
            len(cur_nodes) + 1 > GROUP_NODES or cur_slots(test) > GROUP_SLOTS
        ):
            close_group()
            test = rows
        cur_rows = test
        cur_nodes.append(int(n))
    if cur_nodes:
        close_group()

    slot_tok = []
    for rows in group_rows:
        toks = sorted({tok_of_row[r] for r in rows})
        assert len(toks) <= GROUP_SLOTS
        slot_tok.append(toks)
    return groups, slot_tok, tok_of_row, half_of_row


def _plan(src_idx, dst_idx, att):
    deg = np.bincount(dst_idx, minlength=N_DST)
    # snake-split nodes across cores by degree
    order = np.argsort(-deg, kind="stable")
    core_of_node = np.zeros(N_DST, dtype=np.int64)
    for i, n in enumerate(order):
        blk, pos = divmod(i, N_CORES)
        core_of_node[n] = pos if blk % 2 == 0 else N_CORES - 1 - pos

    core_e = core_of_node[dst_idx]
    plans = []
    for c in range(N_CORES):
        nodes_c = np.flatnonzero((core_of_node == c) & (deg > 0))
        edge_ids_c = np.flatnonzero(core_e == c)
        plans.append(_plan_core(nodes_c, deg, dst_idx, src_idx, edge_ids_c))

    G = max(len(p[0]) for p in plans)
    G = -(-G // 3) * 3
    NB = G * BLOCKS_PER_GROUP
    NS = NB * P

    idx2 = np.zeros((N_CORES, NS), dtype=np.int16)
    a3 = np.zeros((N_CORES, NS, 2 * GROUP_NODES), dtype=np.float32)
    node_at = np.full((N_CORES, G, GROUP_NODES), -1, dtype=np.int64)
    sigma = np.zeros((N_CORES, NPAIR, 2), dtype=np.int64)

    for c in range(N_CORES):
        groups, slot_tok, tok_of_row, half_of_row = plans[c]
        used = np.zeros(NPAIR * 2, dtype=bool)
        rowfill = np.zeros((NPAIR, 2), dtype=np.int64)
        for r, t in tok_of_row.items():
            h = half_of_row[r]
            rowfill[t, h] = r
            used[t * 2 + h] = True
        free_slots = np.flatnonzero(~used)
        allrows = np.ones(NPAIR * 2, dtype=bool)
        refd = np.array(list(tok_of_row.keys()), dtype=np.int64)
        if len(refd):
            allrows[refd] = False
        leftover = np.flatnonzero(allrows)
        ns = min(len(free_slots), len(leftover))
        rowfill.reshape(-1)[free_slots[:ns]] = leftover[:ns]
        sigma[c] = rowfill

        col_of = {}
        for g, members in enumerate(groups):
            for j, n in enumerate(members):
                node_at[c, g, j] = n
                col_of[n] = (g, j)
        slotidx = {}
        for g, toks in enumerate(slot_tok):
            base = g * GROUP_SLOTS
            for k, t in enumerate(toks):
                idx2[c, base + k] = t
                slotidx[(g, t)] = base + k
        eids = np.flatnonzero(core_e == c)
        s_slots = np.empty(len(eids), dtype=np.int64)
        s_cols = np.empty(len(eids), dtype=np.int64)
        for k, e in enumerate(eids):
            r = int(src_idx[e])
            g, j = col_of[int(dst_idx[e])]
            s_slots[k] = slotidx[(g, tok_of_row[r])]
            s_cols[k] = j + GROUP_NODES * half_of_row[r]
        np.add.at(a3[c], (s_slots, s_cols), att[eids])

    chunks = []
    b0 = 0
    while b0 < NB:
        nb = PRIME_BLOCKS if len(chunks) < 4 else CHUNK_BLOCKS
        nb = min(nb, NB - b0)
        chunks.append((b0, nb))
        b0 += nb

    return {
        "NB": NB,
        "G": G,
        "chunks": tuple(chunks),
        "idx2": idx2,
        "a3": a3.astype(np.float16),
        "node_at": node_at,
        "sigma": sigma,
        "pad_frac": 1.0 - len(dst_idx) / (N_CORES * NS),
    }


def _build_nc(NB, chunks):
    import concourse.bacc as bacc
    import concourse.mybir as mybir
    from concourse.tile import TileContext
    from concourse.library_config import mlp

    NS = NB * P
    nsuper = NB // 9

    nc = bacc.Bacc(
        "TRN2", target_bir_lowering=False, debug=False, num_swdge_queues=4,
        dynamic_dma_scratch_size=65536,
    )
    embP = nc.dram_tensor("embP", [NPAIR, P], mybir.dt.float16, kind="ExternalInput")
    idxT = nc.dram_tensor("idxT", [P, NS // 16], mybir.dt.int16, kind="ExternalInput")
    atab = nc.dram_tensor("atab", [P, NB * 64], mybir.dt.float16, kind="ExternalInput")
    out = nc.dram_tensor("out", [nsuper * 96, D], mybir.dt.float32, kind="ExternalOutput")

    with TileContext(nc) as tc:
        nc.gpsimd.load_library(mlp)
        with (
            tc.tile_pool(name="tbl", bufs=1) as tbl,
            tc.tile_pool(name="msg", bufs=12) as msgp,
            tc.tile_pool(name="apool", bufs=8) as apool,
            tc.tile_pool(name="psum", bufs=8, space="PSUM") as psump,
            tc.tile_pool(name="stg", bufs=6) as stgp,
        ):
            head_blocks = sum(nb for _, nb in chunks[:5])
            head_cols = head_blocks * 8
            tail_cols = NS // 16 - head_cols
            idx_a = tbl.tile([P, head_cols], mybir.dt.int16, tag="idxa")
            nc.sync.dma_start(idx_a[:], idxT[:, :head_cols])
            if tail_cols > 0:
                idx_b = tbl.tile([P, tail_cols], mybir.dt.int16, tag="idxb")
                nc.sync.dma_start(idx_b[:], idxT[:, head_cols:])

            psum_tiles = {}
            stage_tiles = {}
            for ci, (b0, nb) in enumerate(chunks):
                q = ci % 4
                c_lo, c_hi = b0 * 8, (b0 + nb) * 8
                if c_hi <= head_cols:
                    iap = idx_a[:, c_lo:c_hi]
                else:
                    iap = idx_b[:, c_lo - head_cols : c_hi - head_cols]
                nidx = nb * P
                msg = msgp.tile([P, CHUNK_BLOCKS, P], mybir.dt.float16, tag="m")
                nc.gpsimd.dma_gather(
                    msg[:, :nb, :], embP[:, :],
                    iap, nidx, nidx, P,
                    transpose=False, single_packet=False, queue_num=q,
                )
                a_t = apool.tile([P, CHUNK_BLOCKS * 64], mybir.dt.float16, tag="a")
                nc.scalar.dma_start(a_t[:, : nb * 64], atab[:, b0 * 64 : (b0 + nb) * 64])

                for j in range(nb):
                    b = b0 + j
                    g = b // BLOCKS_PER_GROUP
                    st = b // (3 * BLOCKS_PER_GROUP)
                    gl = g % 3
                    if g not in psum_tiles:
                        psum_tiles[g] = psump.tile(
                            [64, P], mybir.dt.float32, tag="ps", name=f"ps{g}"
                        )
                    ps = psum_tiles[g]
                    nc.tensor.matmul(
                        ps[:, :], a_t[:, j * 64 : j * 64 + 64], msg[:, j, :],
                        start=(b % BLOCKS_PER_GROUP == 0),
                        stop=(b % BLOCKS_PER_GROUP == BLOCKS_PER_GROUP - 1),
                    )
                    if b % BLOCKS_PER_GROUP == BLOCKS_PER_GROUP - 1:
                        if gl == 0:
                            stage_tiles[st] = stgp.tile(
                                [96, D], mybir.dt.float32, tag="st", name=f"st{st}"
                            )
                        stage = stage_tiles[st]
                        nc.scalar.copy(
                            stage[32 * gl : 32 * gl + 32, :], ps[0:32, 0:D]
                        )
                        nc.vector.tensor_tensor(
                            stage[32 * gl : 32 * gl + 32, :],
                            stage[32 * gl : 32 * gl + 32, :],
                            ps[32:64, D : 2 * D],
                            mybir.AluOpType.add,
                        )
                        del psum_tiles[g]
                        if gl == 2:
                            nc.sync.dma_start(
                                out[st * 96 : (st + 1) * 96, :], stage[:, :]
                            )
                            del stage_tiles[st]
    nc.compile()
    return nc


def plan_and_build(src_idx, dst_idx, e_att):
    src_idx = np.asarray(src_idx, dtype=np.int64)
    dst_idx = np.asarray(dst_idx, dtype=np.int64)
    att_flat = np.asarray(e_att, dtype=np.float32).reshape(-1)
    return _plan(src_idx, dst_idx, att_flat)


def kernel(src_emb, e_att, src_idx, dst_idx):
    from concourse.bass_utils import run_bass_kernel_spmd

    src_emb = np.asarray(src_emb, dtype=np.float32)
    pl = plan_and_build(src_idx, dst_idx, e_att)

    key = (pl["NB"], pl["chunks"])
    if key not in _cache:
        _cache.clear()
        _cache[key] = _build_nc(pl["NB"], pl["chunks"])
    nc = _cache[key]

    emb16 = np.zeros((NPAIR * 2, D), dtype=np.float16)
    emb16[:N_SRC] = src_emb.astype(np.float16)

    NB = pl["NB"]
    in_maps = []
    for c in range(N_CORES):
        embPc = emb16[pl["sigma"][c].reshape(-1)].reshape(NPAIR, P)
        at = np.ascontiguousarray(
            pl["a3"][c].reshape(NB, P, 64).transpose(1, 0, 2).reshape(P, NB * 64)
        )
        in_maps.append(
            {
                "embP": np.ascontiguousarray(embPc),
                "idxT": np.ascontiguousarray(_wrap_idx(pl["idx2"][c].reshape(-1))),
                "atab": at,
            }
        )
    kwargs = {}
    if TRACE:
        kwargs = {"trace": True, "tmpdir": TRACE_DIR}
    res = run_bass_kernel_spmd(nc, in_maps, core_ids=list(range(N_CORES)), **kwargs)
    global LAST_EXEC_NS
    LAST_EXEC_NS = res.exec_time_ns

    out_full = np.zeros((N_DST, D), dtype=np.float32)
    G = pl["G"]
    node_at = pl["node_at"]  # [ncores, G, 32]
    for c in range(N_CORES):
        ids = node_at[c].reshape(-1)
        valid = ids >= 0
        out_full[ids[valid]] = res.results[c]["out"][valid]
    return out_full


# revision 11
# speedup vs baseline: 1.0424x; 1.0424x over previous
"""GNN message passing (src_mul_edge + segment_sum) on 8 Trainium2 cores. v7.

out[n] = sum_{e : dst[e]==n} e_att[e] * src_emb[src[e]]

PE scatter-matmul + per-core token-merge design:
  * Per core, rows are PAIRED by a permutation sigma chosen so that rows
    co-used by the same node-group share one 256-byte pair-token; each
    gathered token then serves edges of BOTH its rows (even/odd halves are
    weighted independently), cutting gather descriptors/bytes ~20%.
  * embP_c [25088, 128] fp16: token t = (sigma_2t | sigma_2t+1) rows.
  * Node groups: <=32 nodes, <=384 post-merge slots (3 blocks of 128);
    uniform 3-block groups -> one shared program for all 8 cores.
  * dma_gather(transpose=False) round-robin over all 4 SWDGE queues.
  * Per 128-slot block one fused matmul on the tensor engine:
      psum[64, 128] += [A_ev | A_od][128, 64].T @ msg[128, 128]
    Valid quadrants: rows 0:32 x cols 0:64 (even halves), rows 32:64 x
    cols 64:128 (odd halves); cross-quadrants are ignored garbage.
    PSUM accumulates over the group's 3 blocks.
  * Supertile = 3 groups: scalar-engine copy + DVE add fold the quadrants
    into stage[96, 64]; one DMA per supertile to DRAM. Host unpermutes.
"""

import numpy as np

N_SRC = 50000
N_DST = 50000
D = 64
N_CORES = 8
P = 128
NPAIR = 25088
GROUP_SLOTS = 384          # 3 blocks of 128
GROUP_NODES = 32           # PSUM strip width
BLOCKS_PER_GROUP = 3
CHUNK_BLOCKS = 16
PRIME_BLOCKS = 4           # first 4 chunks are 4 blocks each

_cache: dict = {}

TRACE = False
TRACE_DIR = None
LAST_EXEC_NS = None


def _wrap_idx(idx_flat):
    w = idx_flat.reshape(-1, 16).T
    return np.tile(w, (8, 1))


def _plan_core(nodes_c, deg, dst_idx, src_idx, edge_ids_c):
    """Group one core's nodes and build its sigma pairing.

    Returns (groups, slot_tok, tok_of_row, half_of_row).
    """
    eo = edge_ids_c[np.argsort(dst_idx[edge_ids_c], kind="stable")]
    ds = dst_idx[eo]
    ss = src_idx[eo]
    starts = np.searchsorted(ds, nodes_c)
    ends = np.searchsorted(ds, nodes_c, side="right")
    node_rows = {int(n): ss[a:b] for n, a, b in zip(nodes_c, starts, ends)}

    nodes_sorted = nodes_c[np.argsort(-deg[nodes_c], kind="stable")]

    tok_of_row = {}
    half_of_row = {}
    state = {"n_tok": 0, "pending": None}

    seen = set()
    groups = []
    group_rows = []
    cur_rows = set()
    cur_nodes = []

    def cur_slots(rows):
        ext = sum(1 for r in rows if r in seen)
        fa = len(rows) - ext
        return ext + (fa + 1) // 2

    def close_group():
        nonlocal cur_rows, cur_nodes
        fa = [r for r in cur_rows if r not in seen]
        i = 0
        while i + 1 < len(fa):
            r1, r2 = fa[i], fa[i + 1]
            tok_of_row[r1] = state["n_tok"]
            half_of_row[r1] = 0
            tok_of_row[r2] = state["n_tok"]
            half_of_row[r2] = 1
            state["n_tok"] += 1
            i += 2
        if i < len(fa):
            r = fa[i]
            if state["pending"] is not None:
                tok_of_row[r] = tok_of_row[state["pending"]]
                half_of_row[r] = 1
                state["pending"] = None
            else:
                tok_of_row[r] = state["n_tok"]
                half_of_row[r] = 0
                state["pending"] = r
                state["n_tok"] += 1
        seen.update(fa)
        groups.append(cur_nodes)
        group_rows.append(cur_rows)
        cur_rows = set()
        cur_nodes = []

    for n in nodes_sorted:
        rows = set(int(r) for r in node_rows[int(n)])
        test = cur_rows | rows
        if cur_nodes and (
            len(cur_nodes) + 1 > GROUP_NODES or cur_slots(test) > GROUP_SLOTS
        ):
            close_group()
            test = rows
        cur_rows = test
        cur_nodes.append(int(n))
    if cur_nodes:
        close_group()

    slot_tok = []
    for rows in group_rows:
        toks = sorted({tok_of_row[r] for r in rows})
        assert len(toks) <= GROUP_SLOTS
        slot_tok.append(toks)
    return groups, slot_tok, tok_of_row, half_of_row


def _plan(src_idx, dst_idx, att):
    deg = np.bincount(dst_idx, minlength=N_DST)
    # snake-split nodes across cores by degree
    order = np.argsort(-deg, kind="stable")
    core_of_node = np.zeros(N_DST, dtype=np.int64)
    for i, n in enumerate(order):
        blk, pos = divmod(i, N_CORES)
        core_of_node[n] = pos if blk % 2 == 0 else N_CORES - 1 - pos

    core_e = core_of_node[dst_idx]
    plans = []
    for c in range(N_CORES):
        nodes_c = np.flatnonzero((core_of_node == c) & (deg > 0))
        edge_ids_c = np.flatnonzero(core_e == c)
        plans.append(_plan_core(nodes_c, deg, dst_idx, src_idx, edge_ids_c))

    G = max(len(p[0]) for p in plans)
    G = -(-G // 3) * 3
    NB = G * BLOCKS_PER_GROUP
    NS = NB * P

    idx2 = np.zeros((N_CORES, NS), dtype=np.int16)
    a3 = np.zeros((N_CORES, NS, 2 * GROUP_NODES), dtype=np.float32)
    node_at = np.full((N_CORES, G, GROUP_NODES), -1, dtype=np.int64)
    sigma = np.zeros((N_CORES, NPAIR, 2), dtype=np.int64)

    for c in range(N_CORES):
        groups, slot_tok, tok_of_row, half_of_row = plans[c]
        used = np.zeros(NPAIR * 2, dtype=bool)
        rowfill = np.zeros((NPAIR, 2), dtype=np.int64)
        for r, t in tok_of_row.items():
            h = half_of_row[r]
            rowfill[t, h] = r
            used[t * 2 + h] = True
        free_slots = np.flatnonzero(~used)
        allrows = np.ones(NPAIR * 2, dtype=bool)
        refd = np.array(list(tok_of_row.keys()), dtype=np.int64)
        if len(refd):
            allrows[refd] = False
        leftover = np.flatnonzero(allrows)
        ns = min(len(free_slots), len(leftover))
        rowfill.reshape(-1)[free_slots[:ns]] = leftover[:ns]
        sigma[c] = rowfill

        col_of = {}
        for g, members in enumerate(groups):
            for j, n in enumerate(members):
                node_at[c, g, j] = n
                col_of[n] = (g, j)
        slotidx = {}
        for g, toks in enumerate(slot_tok):
            base = g * GROUP_SLOTS
            for k, t in enumerate(toks):
                idx2[c, base + k] = t
                slotidx[(g, t)] = base + k
        eids = np.flatnonzero(core_e == c)
        s_slots = np.empty(len(eids), dtype=np.int64)
        s_cols = np.empty(len(eids), dtype=np.int64)
        for k, e in enumerate(eids):
            r = int(src_idx[e])
            g, j = col_of[int(dst_idx[e])]
            s_slots[k] = slotidx[(g, tok_of_row[r])]
            s_cols[k] = j + GROUP_NODES * half_of_row[r]
        np.add.at(a3[c], (s_slots, s_cols), att[eids])

    chunks = []
    b0 = 0
    while b0 < NB:
        nb = PRIME_BLOCKS if len(chunks) < 4 else CHUNK_BLOCKS
        nb = min(nb, NB - b0)
        chunks.append((b0, nb))
        b0 += nb

    return {
        "NB": NB,
        "G": G,
        "chunks": tuple(chunks),
        "idx2": idx2,
        "a3": a3.astype(np.float16),
        "node_at": node_at,
        "sigma": sigma,
        "pad_frac": 1.0 - len(dst_idx) / (N_CORES * NS),
    }


def _build_nc(NB, chunks):
    import concourse.bacc as bacc
    import concourse.mybir as mybir
    from concourse.tile import TileContext
    from concourse.library_config import mlp

    NS = NB * P
    nsuper = NB // 9

    nc = bacc.Bacc(
        "TRN2", target_bir_lowering=False, debug=False, num_swdge_queues=4,
        dynamic_dma_scratch_size=65536,
    )
    embP = nc.dram_tensor("embP", [NPAIR, P], mybir.dt.float16, kind="ExternalInput")
    idxT = nc.dram_tensor("idxT", [P, NS // 16], mybir.dt.int16, kind="ExternalInput")
    atab = nc.dram_tensor("atab", [P, NB * 64], mybir.dt.float16, kind="ExternalInput")
    out = nc.dram_tensor("out", [nsuper * 96, D], mybir.dt.float32, kind="ExternalOutput")

    with TileContext(nc) as tc:
        nc.gpsimd.load_library(mlp)
        with (
            tc.tile_pool(name="tbl", bufs=1) as tbl,
            tc.tile_pool(name="msg", bufs=12) as msgp,
            tc.tile_pool(name="apool", bufs=8) as apool,
            tc.tile_pool(name="psum", bufs=8, space="PSUM") as psump,
            tc.tile_pool(name="stg", bufs=6) as stgp,
        ):
            head_blocks = sum(nb for _, nb in chunks[:5])
            head_cols = head_blocks * 8
            tail_cols = NS // 16 - head_cols
            idx_a = tbl.tile([P, head_cols], mybir.dt.int16, tag="idxa")
            nc.sync.dma_start(idx_a[:], idxT[:, :head_cols])
            if tail_cols > 0:
                idx_b = tbl.tile([P, tail_cols], mybir.dt.int16, tag="idxb")
                nc.sync.dma_start(idx_b[:], idxT[:, head_cols:])

            psum_tiles = {}
            stage_tiles = {}
            for ci, (b0, nb) in enumerate(chunks):
                q = ci % 4
                c_lo, c_hi = b0 * 8, (b0 + nb) * 8
                if c_hi <= head_cols:
                    iap = idx_a[:, c_lo:c_hi]
                else:
                    iap = idx_b[:, c_lo - head_cols : c_hi - head_cols]
                nidx = nb * P
                msg = msgp.tile([P, CHUNK_BLOCKS, P], mybir.dt.float16, tag="m")
                nc.gpsimd.dma_gather(
                    msg[:, :nb, :], embP[:, :],
                    iap, nidx, nidx, P,
                    transpose=False, single_packet=False, queue_num=q,
                )
                a_t = apool.tile([P, CHUNK_BLOCKS * 64], mybir.dt.float16, tag="a")
                nc.scalar.dma_start(a_t[:, : nb * 64], atab[:, b0 * 64 : (b0 + nb) * 64])

                for j in range(nb):
                    b = b0 + j
                    g = b // BLOCKS_PER_GROUP
                    st = b // (3 * BLOCKS_PER_GROUP)
                    gl = g % 3
                    gp, ga = g // 2, 64 * (g % 2)
                    if gp not in psum_tiles:
                        psum_tiles[gp] = psump.tile(
                            [P, P], mybir.dt.float32, tag="ps", name=f"ps{gp}"
                        )
                    ps = psum_tiles[gp]
                    nc.tensor.matmul(
                        ps[ga : ga + 64, :], a_t[:, j * 64 : j * 64 + 64],
                        msg[:, j, :],
                        start=(b % BLOCKS_PER_GROUP == 0),
                        stop=(b % BLOCKS_PER_GROUP == BLOCKS_PER_GROUP - 1),
                    )
                    if b % BLOCKS_PER_GROUP == BLOCKS_PER_GROUP - 1:
                        if gl == 0:
                            stage_tiles[st] = stgp.tile(
                                [96, D], mybir.dt.float32, tag="st", name=f"st{st}"
                            )
                        stage = stage_tiles[st]
                        nc.vector.tensor_copy(
                            stage[32 * gl : 32 * gl + 32, :],
                            ps[ga : ga + 32, 0:D],
                        )
                        nc.vector.tensor_tensor(
                            stage[32 * gl : 32 * gl + 32, :],
                            stage[32 * gl : 32 * gl + 32, :],
                            ps[ga + 32 : ga + 64, D : 2 * D],
                            mybir.AluOpType.add,
                        )
                        if g % 2 == 1 or g == NB // BLOCKS_PER_GROUP - 1:
                            del psum_tiles[gp]
                        if gl == 2:
                            nc.sync.dma_start(
                                out[st * 96 : (st + 1) * 96, :], stage[:, :]
                            )
                            del stage_tiles[st]
    nc.compile()
    return nc


def plan_and_build(src_idx, dst_idx, e_att):
    src_idx = np.asarray(src_idx, dtype=np.int64)
    dst_idx = np.asarray(dst_idx, dtype=np.int64)
    att_flat = np.asarray(e_att, dtype=np.float32).reshape(-1)
    return _plan(src_idx, dst_idx, att_flat)


def kernel(src_emb, e_att, src_idx, dst_idx):
    from concourse.bass_utils import run_bass_kernel_spmd

    src_emb = np.asarray(src_emb, dtype=np.float32)
    pl = plan_and_build(src_idx, dst_idx, e_att)

    key = (pl["NB"], pl["chunks"])
    if key not in _cache:
        _cache.clear()
        _cache[key] = _build_nc(pl["NB"], pl["chunks"])
    nc = _cache[key]

    emb16 = np.zeros((NPAIR * 2, D), dtype=np.float16)
    emb16[:N_SRC] = src_emb.astype(np.float16)

    NB = pl["NB"]
    in_maps = []
    for c in range(N_CORES):
        embPc = emb16[pl["sigma"][c].reshape(-1)].reshape(NPAIR, P)
        at = np.ascontiguousarray(
            pl["a3"][c].reshape(NB, P, 64).transpose(1, 0, 2).reshape(P, NB * 64)
        )
        in_maps.append(
            {
                "embP": np.ascontiguousarray(embPc),
                "idxT": np.ascontiguousarray(_wrap_idx(pl["idx2"][c].reshape(-1))),
                "atab": at,
            }
        )
    kwargs = {}
    if TRACE:
        kwargs = {"trace": True, "tmpdir": TRACE_DIR}
    res = run_bass_kernel_spmd(nc, in_maps, core_ids=list(range(N_CORES)), **kwargs)
    global LAST_EXEC_NS
    LAST_EXEC_NS = res.exec_time_ns

    out_full = np.zeros((N_DST, D), dtype=np.float32)
    G = pl["G"]
    node_at = pl["node_at"]  # [ncores, G, 32]
    for c in range(N_CORES):
        ids = node_at[c].reshape(-1)
        valid = ids >= 0
        out_full[ids[valid]] = res.results[c]["out"][valid]
    return out_full


# revision 12
# speedup vs baseline: 1.0958x; 1.0513x over previous
"""GNN message passing (src_mul_edge + segment_sum) on 8 Trainium2 cores. v6.

out[n] = sum_{e : dst[e]==n} e_att[e] * src_emb[src[e]]

PE scatter-matmul design:
  * src_emb rows cast to fp16; consecutive row PAIRS form 256-byte tokens in
    DRAM ([25088, 128] fp16); token ids fit int16 (single index window).
  * Nodes bin-packed into GROUPS: <=32 nodes, total degree <= 512 slots
    (4 blocks of 128). Groups uniform across cores -> one shared program.
  * Edge slots: per group, edges sorted by token; pad slots use token 0 with
    A = 0. dma_gather(transpose=False) -> msg[slot%128, block, 0:128] fp16,
    round-robin over all 4 SWDGE queues (8 Q7 descriptor cores).
  * Per 128-slot block: two matmuls on the (otherwise idle) tensor engine:
      psum[32 nodes, 64] += A_ev[128,32].T @ msg[:,b,0:64]
                          + A_od[128,32].T @ msg[:,b,64:128]
    where A_ev/A_od hold att for even/odd-half edges (0 elsewhere) and
    PSUM accumulates over the group's 4 blocks.
  * Supertile = 4 groups = one PSUM tile [128, 64] fp32; DVE evacuates to
    SBUF, single DMA to DRAM out. Host unpermutes rows.
"""

import numpy as np

N_SRC = 50000
N_DST = 50000
D = 64
N_CORES = 8
P = 128
NPAIR = 25088
GROUP_SLOTS = 512          # 4 blocks of 128
GROUP_NODES = 32           # PSUM strip width
CHUNK_BLOCKS = 16          # gather chunk = 16 blocks = 2048 slots
PRIME_BLOCKS = 4           # first 4 chunks are 4 blocks each

_cache: dict = {}

TRACE = False
TRACE_DIR = None
LAST_EXEC_NS = None


def _wrap_idx(idx_flat):
    w = idx_flat.reshape(-1, 16).T
    return np.tile(w, (8, 1))


def _binpack(deg):
    """Pack nodes into groups: <=GROUP_NODES nodes, sum(deg) <= GROUP_SLOTS.
    Picks the available degree closest to the remaining per-node target so the
    degree mix stays balanced through the tail. Returns list of node-id lists."""
    maxd = int(deg.max())
    order = np.argsort(-deg, kind="stable")
    nodes_by_deg = [[] for _ in range(maxd + 1)]
    for n in order:
        d = deg[n]
        if d > 0:
            nodes_by_deg[d].append(int(n))
    ptr = [0] * (maxd + 1)
    avail = [len(nodes_by_deg[d]) - ptr[d] for d in range(maxd + 1)]
    remaining = sum(avail[1:])
    groups = []
    while remaining > 0:
        cap = GROUP_SLOTS
        members = []
        while len(members) < GROUP_NODES and cap > 0:
            tgt = cap / (GROUP_NODES - len(members))
            best = -1
            bestdist = None
            d = min(cap, maxd)
            while d >= 1:
                if avail[d] > 0:
                    dist = abs(d - tgt)
                    if bestdist is None or dist < bestdist:
                        bestdist = dist
                        best = d
                    elif d < tgt and dist > bestdist:
                        break
                d -= 1
            if best < 0:
                break
            members.append(nodes_by_deg[best][ptr[best]])
            ptr[best] += 1
            avail[best] -= 1
            remaining -= 1
            cap -= best
        groups.append(members)
    return groups


def _plan(dst_idx, tok, half, att):
    deg = np.bincount(dst_idx, minlength=N_DST)
    groups = _binpack(deg)
    ng = len(groups)
    # per-core group count: multiple of 3 (supertile = 3 groups / 96 psum rows)
    G = -(-ng // N_CORES)
    G = -(-G // 3) * 3
    ng_pad = G * N_CORES
    NB = G * 4                      # blocks per core
    NS = NB * P                     # slots per core

    # group id (global, 0..ng_pad), col within group for every node
    grp_of = np.full(N_DST, -1, dtype=np.int64)
    col_of = np.full(N_DST, -1, dtype=np.int64)
    node_at = np.full((ng_pad, GROUP_NODES), -1, dtype=np.int64)
    for g, members in enumerate(groups):
        m = np.asarray(members, dtype=np.int64)
        grp_of[m] = g
        col_of[m] = np.arange(len(m))
        node_at[g, : len(m)] = m

    E = len(dst_idx)
    g_e = grp_of[dst_idx]
    assert (g_e >= 0).all()
    eorder = np.lexsort((tok, g_e))
    g_s = g_e[eorder]
    # rank of each edge within its group
    gstart = np.searchsorted(g_s, np.arange(ng + 1))
    rank = np.arange(E) - gstart[g_s]
    slot_global = g_s * GROUP_SLOTS + rank      # 0 .. ng*512
    core_e = slot_global // (G * GROUP_SLOTS)
    slot_e = slot_global % (G * GROUP_SLOTS)

    # per-core tables
    idx2 = np.zeros((N_CORES, NS), dtype=np.int16)           # token per slot
    a3 = np.zeros((N_CORES, NS, 2 * GROUP_NODES), dtype=np.float16)
    col_e = col_of[dst_idx][eorder]
    half_e = half[eorder]
    idx2[core_e, slot_e] = tok[eorder]
    a3[core_e, slot_e, col_e + GROUP_NODES * half_e] = att[eorder]

    # chunk schedule (blocks): 4 priming chunks, then CHUNK_BLOCKS
    chunks = []
    b0 = 0
    while b0 < NB:
        nb = PRIME_BLOCKS if len(chunks) < 4 else CHUNK_BLOCKS
        nb = min(nb, NB - b0)
        chunks.append((b0, nb))
        b0 += nb

    return {
        "NB": NB,
        "G": G,
        "chunks": tuple(chunks),
        "idx2": idx2,
        "a3": a3,
        "node_at": node_at,
        "pad_frac": 1.0 - E / (ng_pad * GROUP_SLOTS),
    }


def _build_nc(NB, chunks):
    import concourse.bacc as bacc
    import concourse.mybir as mybir
    from concourse.tile import TileContext
    from concourse.library_config import mlp

    NS = NB * P
    nsuper = NB // 12

    nc = bacc.Bacc(
        "TRN2", target_bir_lowering=False, debug=False, num_swdge_queues=4,
        dynamic_dma_scratch_size=65536,
    )
    embP = nc.dram_tensor("embP", [NPAIR, P], mybir.dt.float16, kind="ExternalInput")
    idxT = nc.dram_tensor("idxT", [P, NS // 16], mybir.dt.int16, kind="ExternalInput")
    atab = nc.dram_tensor("atab", [P, NB * 64], mybir.dt.float16, kind="ExternalInput")
    out = nc.dram_tensor("out", [nsuper * 96, D], mybir.dt.float32, kind="ExternalOutput")

    with TileContext(nc) as tc:
        nc.gpsimd.load_library(mlp)
        with (
            tc.tile_pool(name="tbl", bufs=1) as tbl,
            tc.tile_pool(name="msg", bufs=12) as msgp,
            tc.tile_pool(name="apool", bufs=8) as apool,
            tc.tile_pool(name="psum", bufs=8, space="PSUM") as psump,
            tc.tile_pool(name="stg", bufs=6) as stgp,
        ):
            # two-stage idx load: head slice unblocks the first gathers
            head_blocks = sum(nb for _, nb in chunks[:5])
            head_cols = head_blocks * 8
            tail_cols = NS // 16 - head_cols
            idx_a = tbl.tile([P, head_cols], mybir.dt.int16, tag="idxa")
            nc.sync.dma_start(idx_a[:], idxT[:, :head_cols])
            if tail_cols > 0:
                idx_b = tbl.tile([P, tail_cols], mybir.dt.int16, tag="idxb")
                nc.sync.dma_start(idx_b[:], idxT[:, head_cols:])

            psum_tiles = {}
            stage_tiles = {}
            for ci, (b0, nb) in enumerate(chunks):
                q = ci % 4
                c_lo, c_hi = b0 * 8, (b0 + nb) * 8
                if c_hi <= head_cols:
                    iap = idx_a[:, c_lo:c_hi]
                else:
                    iap = idx_b[:, c_lo - head_cols : c_hi - head_cols]
                nidx = nb * P
                msg = msgp.tile([P, CHUNK_BLOCKS, P], mybir.dt.float16, tag="m")
                nc.gpsimd.dma_gather(
                    msg[:, :nb, :], embP[:, :],
                    iap, nidx, nidx, P,
                    transpose=False, single_packet=False, queue_num=q,
                )
                a_t = apool.tile([P, CHUNK_BLOCKS * 64], mybir.dt.float16, tag="a")
                nc.scalar.dma_start(a_t[:, : nb * 64], atab[:, b0 * 64 : (b0 + nb) * 64])

                for j in range(nb):
                    b = b0 + j
                    g = b // 4                   # group id
                    st = b // 12
                    gl = g % 3                   # group within supertile
                    gp, ga = g // 2, 64 * (g % 2)
                    if gp not in psum_tiles:
                        # [128, 128]: two groups stacked; per group the
                        # quadrant trick — rows +0:32 even part (valid cols
                        # 0:64), rows +32:64 odd part (cols 64:128)
                        psum_tiles[gp] = psump.tile(
                            [P, P], mybir.dt.float32, tag="ps", name=f"ps{gp}"
                        )
                    ps = psum_tiles[gp]
                    nc.tensor.matmul(
                        ps[ga : ga + 64, :], a_t[:, j * 64 : j * 64 + 64],
                        msg[:, j, :],
                        start=(b % 4 == 0), stop=(b % 4 == 3),
                    )
                    if b % 4 == 3:
                        if gl == 0:
                            stage_tiles[st] = stgp.tile(
                                [96, D], mybir.dt.float32, tag="st", name=f"st{st}"
                            )
                        stage = stage_tiles[st]
                        nc.vector.tensor_copy(
                            stage[32 * gl : 32 * gl + 32, :],
                            ps[ga : ga + 32, 0:D],
                        )
                        nc.vector.tensor_tensor(
                            stage[32 * gl : 32 * gl + 32, :],
                            stage[32 * gl : 32 * gl + 32, :],
                            ps[ga + 32 : ga + 64, D : 2 * D],
                            mybir.AluOpType.add,
                        )
                        if g % 2 == 1 or g == NB // 4 - 1:
                            del psum_tiles[gp]
                        if gl == 2:
                            nc.sync.dma_start(
                                out[st * 96 : (st + 1) * 96, :], stage[:, :]
                            )
                            del stage_tiles[st]
    nc.compile()
    return nc


def plan_and_build(src_idx, dst_idx, e_att):
    src_idx = np.asarray(src_idx, dtype=np.int64)
    dst_idx = np.asarray(dst_idx, dtype=np.int64)
    att_flat = np.asarray(e_att, dtype=np.float16).reshape(-1)
    tok = (src_idx // 2).astype(np.int16)
    half = (src_idx & 1).astype(np.int64)
    return _plan(dst_idx, tok, half, att_flat)


def kernel(src_emb, e_att, src_idx, dst_idx):
    from concourse.bass_utils import run_bass_kernel_spmd

    src_emb = np.asarray(src_emb, dtype=np.float32)
    pl = plan_and_build(src_idx, dst_idx, e_att)

    key = (pl["NB"], pl["chunks"])
    if key not in _cache:
        _cache.clear()
        _cache[key] = _build_nc(pl["NB"], pl["chunks"])
    nc = _cache[key]

    embP = np.zeros((NPAIR * 2, D), dtype=np.float16)
    embP[:N_SRC] = src_emb.astype(np.float16)
    embP = np.ascontiguousarray(embP.reshape(NPAIR, P))

    NB = pl["NB"]
    in_maps = []
    for c in range(N_CORES):
        # atab layout: [128 (slot in block), NB*64] fp16
        at = np.ascontiguousarray(
            pl["a3"][c].reshape(NB, P, 64).transpose(1, 0, 2).reshape(P, NB * 64)
        )
        in_maps.append(
            {
                "embP": embP,
                "idxT": np.ascontiguousarray(_wrap_idx(pl["idx2"][c].reshape(-1))),
                "atab": at,
            }
        )
    kwargs = {}
    if TRACE:
        kwargs = {"trace": True, "tmpdir": TRACE_DIR}
    res = run_bass_kernel_spmd(nc, in_maps, core_ids=list(range(N_CORES)), **kwargs)
    global LAST_EXEC_NS
    LAST_EXEC_NS = res.exec_time_ns

    out_full = np.zeros((N_DST, D), dtype=np.float32)
    G = pl["G"]
    node_at = pl["node_at"]  # [ng_pad, 32]
    for c in range(N_CORES):
        ids = node_at[c * G : (c + 1) * G].reshape(-1)   # supertile-row order
        valid = ids >= 0
        out_full[ids[valid]] = res.results[c]["out"][valid]
    return out_full
